# revision 32
# baseline (speedup 1.0000x reference)
"""LeNet C3 grouped-conv layer as a Trainium2 Bass/Tile kernel (bf16, v2).

Math: y[b,o,h,w] = sum_{c,dy,dx} W[o,c,dy,dx] * x[b,c,h+dy,w+dx] + bias[o]
with W the dense 16x6x5x5 weight built from the C3 connectivity tables
(absent connections are zero).

v2 mapping ("3-matmul" scheme; per core, 16 images of the batch):
  - All matmul operands are bf16 (tolerance 2e-2; bf16 gives ~3e-3),
    halving both HBM streams vs f32.
  - The 5 conv column taps are covered by 3 matmuls instead of 5: the
    input tile holds TWO copies of x, the second shifted by +2 columns
    (partitions p = j*60 + c*10 + dr, j in {0,1}).  Matmul dxi shifts
    the moving AP by dxi cols, so tap dx = 2j + dxi; lhsT routes each
    dx to exactly one (j, dxi).  PE cost drops 20% vs the 5-tap scheme
    (3 matmuls per 6 output rows instead of 5 per 8).
  - Row blocks of R=10 input rows yield S=6 output rows (M = 16o*6s =
    96, lhsT band selects dy = dr - s).  42 blocks of stride 6 tile the
    252 output rows EXACTLY (input rows 6k..6k+9, k=0..41).
  - Tiles span all 16 images (cols = img*256 + w, 4096 cols).  Groups
    g = 0..7 cover image pairs with N=512 (g<7) / 508 (g=7) matmul
    columns; PSUM tiles hold 2 groups (2 banks).
  - j=1 copy is built on-chip by ONE VectorE tensor_copy per block
    (bf16 SBUF 4x mode, +2 col = 4B aligned).
  - PSUM evacuation (bias add + bf16 cast) alternates VectorE
    tensor_scalar_add / ScalarE activation(Identity,bias) per 2-group
    op so neither engine bottlenecks.
  - ONE input DMA per block (SWDGE, 480 KB, src x[c, 6k..6k+9, :, :])
    and ONE output DMA per block (HWDGE, 786 KB, dst y[:, 6k:6k+6, :, :]
    with device layout [o, h, img, w=256]: 8 KB contiguous chunks).
    Host transposes y to [img, o, h, w], slices w<252, casts f32.
"""

import os
import sys

sys.path.insert(0, "/opt/trn_rl_repo")

import numpy as np

_CH3 = np.array([[0, 1, 2], [1, 2, 3], [2, 3, 4], [3, 4, 5], [0, 4, 5], [0, 1, 5]])
_CH4 = np.array(
    [
        [0, 1, 2, 3],
        [1, 2, 3, 4],
        [2, 3, 4, 5],
        [0, 3, 4, 5],
        [0, 1, 4, 5],
        [0, 1, 2, 5],
        [0, 1, 3, 4],
        [1, 2, 4, 5],
        [0, 2, 3, 5],
    ]
)
_CH6 = np.array([[0, 1, 2, 3, 4, 5]])

_B_PER_CORE = 16  # 128 batch / 8 cores
_N_CORES = 8
_H = 256
_W = 256
_HO = 252
_WO = 252
_R = 10  # input rows per block
_S = 6  # output rows per block
_NBLK = 42  # 42*6 = 252 output rows, input rows 6k..6k+9 (max 255)
# partitions: j=0 block at 0..59, 4-partition hole at 60..63 (zero lhsT
# rows; filled with real x data so no NaN*0), j=1 block at 64..123 --
# the j=1 DVE copy needs a 32-aligned base partition.
_K = 124  # contraction partitions incl. hole
_M = 16 * _S  # (o, s) = 96 output partitions
_NC = _B_PER_CORE * _W  # 4096 tile columns

_module_cache = {}


def _bf16(a):
    import ml_dtypes

    return np.ascontiguousarray(np.asarray(a, np.float32).astype(ml_dtypes.bfloat16))


def _dense_weights(w3, b3, w4, b4, w6, b6):
    W = np.zeros((16, 6, 5, 5), np.float32)
    bias = np.zeros((16,), np.float32)
    for i in range(6):
        W[i, _CH3[i]] = w3[i]
    bias[0:6] = b3
    for i in range(9):
        W[6 + i, _CH4[i]] = w4[i]
    bias[6:15] = b4
    W[15, _CH6[0]] = w6[0]
    bias[15] = np.asarray(b6).reshape(-1)[0]
    return W, bias


def _host_tensors(w3, b3, w4, b4, w6, b6):
    W, bias = _dense_weights(w3, b3, w4, b4, w6, b6)
    # lhsT[j*64 + c*10 + dr, (dxi,o,s)] = W[o, c, dr-s, 2j+dxi], each tap
    # dx routed to exactly one (j, dxi): j=0 -> dx 0,1,2; j=1 -> dx 3,4.
    # Rows 60..63 (the hole) stay zero.
    lhsT = np.zeros((_K, 3, 16, _S), np.float32)
    for j in range(2):
        for c in range(6):
            for dr in range(_R):
                p = j * 64 + c * 10 + dr
                for dxi in range(3):
                    dx = 2 * j + dxi
                    if j == 1 and dxi == 0:
                        continue  # dx=2 already covered by (j=0, dxi=2)
                    for s in range(_S):
                        dy = dr - s
                        if 0 <= dy < 5:
                            lhsT[p, dxi, :, s] = W[:, c, dy, dx]
    lhsT = np.ascontiguousarray(lhsT.reshape(_K, 3 * _M))
    biasv = np.repeat(bias, _S).reshape(_M, 1).astype(np.float32)  # p = o*6+s
    return lhsT, biasv


def _build_module(reps=1):
    if ("nc", reps) in _module_cache:
        return _module_cache[("nc", reps)]

    import concourse.bacc as bacc
    import concourse.mybir as mybir
    from concourse.tile import TileContext

    f32 = mybir.dt.float32
    bf16 = mybir.dt.bfloat16

    nc = bacc.Bacc("TRN2", target_bir_lowering=False, debug=False)
    # device input layout: (c, h, img, w) -> a block slice is 3-dim with
    # (img, w) one contiguous 4096-col dim.
    x = nc.dram_tensor("x", [6, _H, _B_PER_CORE, _W], bf16, kind="ExternalInput").ap()
    lhsT = nc.dram_tensor("lhsT", [_K, 3 * _M], bf16, kind="ExternalInput").ap()
    biasv = nc.dram_tensor("biasv", [_M, 1], f32, kind="ExternalInput").ap()
    # device output layout: (o, h, img, w)
    y = nc.dram_tensor(
        "y", [16, _HO, _B_PER_CORE, _W], bf16, kind="ExternalOutput"
    ).ap()

    from concourse.tile import add_dep_helper

    with TileContext(nc) as tc:
        with (
            tc.tile_pool(name="const", bufs=1) as cpool,
            tc.tile_pool(name="xin", bufs=6) as xpool,
            tc.tile_pool(name="oup", bufs=3) as opool,
            tc.tile_pool(name="psum", bufs=4, space="PSUM") as ppool,
        ):
            wt = cpool.tile([_K, 3 * _M], bf16)
            nc.sync.dma_start(out=wt, in_=lhsT)
            bv = cpool.tile([_M, 1], f32)
            nc.sync.dma_start(out=bv, in_=biasv)

            xts = {}
            half = _NC // 2

            def emit_input(blk):
                r0 = 6 * (blk % _NBLK)
                xt = xpool.tile([_K, _NC], bf16, tag="xt")
                xts[blk] = xt
                # HWDGE (Sync ring) so DVE 4x copies can't starve
                # SWDGE descriptor generation; issued ahead of out(k) in
                # the Sync FIFO, waiting only on an old xt slot.
                # j=0 at partitions 0..59; hole 60..63 gets real x data
                # (zero lhsT rows) so no NaN*0.
                nc.sync.dma_start(
                    out=xt[0:60, :], in_=x[:, r0 : r0 + _R, :, :]
                )
                nc.sync.dma_start(
                    out=xt[60:64, :], in_=x[0:1, r0 : r0 + 4, :, :]
                )

            def emit_copies(blk, dep0, dep1):
                # j=1: x shifted by +2 cols (4B-aligned -> DVE 4x), in
                # halves; each ordered AFTER a prior-block DVE evac so
                # copies never head-of-line-block PSUM draining.
                xt = xts[blk]
                c0 = nc.vector.tensor_copy(
                    out=xt[64:124, 0 : half - 2], in_=xt[0:60, 2:half]
                )
                c1 = nc.vector.tensor_copy(
                    out=xt[64:124, half - 2 : _NC - 2], in_=xt[0:60, half:_NC]
                )
                if dep0 is not None:
                    add_dep_helper(
                        c0.ins, dep0.ins, sync=False, reason="copy after evac"
                    )
                if dep1 is not None:
                    add_dep_helper(
                        c1.ins, dep1.ins, sync=False, reason="copy after evac"
                    )

            ctr = 0

            def emit_compute(blk):
                nonlocal ctr
                r0 = 6 * (blk % _NBLK)
                xt = xts.pop(blk)
                ot = opool.tile([_M, _NC], bf16, tag="ot")
                dve_evacs = []
                for gp in range(4):  # psum pairs: groups (2gp, 2gp+1)
                    ps = ppool.tile([_M, 1024], f32, tag="ps")
                    for gh in range(2):
                        g = 2 * gp + gh
                        N = 512 if g < 7 else 508
                        for dxi in range(3):
                            nc.tensor.matmul(
                                ps[0:_M, 512 * gh : 512 * gh + N],
                                wt[:, dxi * _M : (dxi + 1) * _M],
                                xt[:, 512 * g + dxi : 512 * g + dxi + N],
                                start=(dxi == 0),
                                stop=(dxi == 2),
                            )
                    lo = 1024 * gp
                    width = 1024 if gp < 3 else 1020
                    if gp == 0:
                        # DVE takes only gp0 (it gates the next block's
                        # first matmuls); ACT absorbs the other three so
                        # the DVE chain [ev0, copy0, copy1] fits a block
                        # with ~2.5us slack.
                        ev = nc.vector.tensor_scalar_add(
                            ot[:, lo : lo + width], ps[0:_M, 0:width], bv
                        )
                        dve_evacs.append(ev)
                    else:
                        nc.scalar.add(
                            ot[:, lo : lo + width], ps[0:_M, 0:width], bv
                        )
                    ctr += 1
                    if gp == 3:
                        nc.vector.memset(ot[:, _NC - 4 : _NC], 0)
                    # per-pair output DMA: issued right after this pair's
                    # evacuation, so Sync issues never sit on long waits
                    # that would head-of-line-block the input issues.
                    nc.sync.dma_start(
                        out=y[:, r0 : r0 + _S, 4 * gp : 4 * gp + 4, :],
                        in_=ot[:, lo : lo + 1024],
                    )
                return dve_evacs

            # PE warm-up: ~3.5us of dummy matmuls (reading the weight
            # tile as both operands) so HAM reaches K=8/8 while the
            # first input tiles are still streaming in.
            wps = ppool.tile([_M, 1024], f32, tag="ps")
            for wi in range(30):
                nc.tensor.matmul(
                    wps[0:_M, 0:288],
                    wt[:, 0:_M],
                    wt,
                    start=(wi == 0),
                    stop=(wi == 29),
                )

            # software pipeline: input DMAs lead compute by LEAD blocks;
            # j=1 copies for block k+1 run between block k's DVE evacs.
            LEAD = 3
            for rep in range(reps):
                base = rep * _NBLK
                if rep == 0:
                    for i in range(LEAD):
                        emit_input(base + i)
                    emit_copies(base, None, None)
                for k in range(_NBLK):
                    if k + LEAD < _NBLK or rep + 1 < reps:
                        emit_input(base + k + LEAD)
                    evs = emit_compute(base + k)
                    if k + 1 < _NBLK or rep + 1 < reps:
                        d0 = evs[0] if len(evs) > 0 else None
                        emit_copies(base + k + 1, d0, d0)

    nc.compile()
    _module_cache[("nc", reps)] = nc
    return nc


def _run(inputs, trace=False, reps=1):
    # Tracing requires the NTFF hook (installed by test.py); default off.
    if not trace:
        os.environ["BASS_NEVER_TRACE"] = "1"
    else:
        os.environ.pop("BASS_NEVER_TRACE", None)

    from concourse.bass_utils import run_bass_kernel_spmd

    # host: cast to bf16 and transpose to the device layout (c, h, img, w)
    x = _bf16(np.asarray(inputs["x"], dtype=np.float32))
    x = x.transpose(1, 2, 0, 3)  # [6, 256, 128, 256]
    lhsT, biasv = _host_tensors(
        np.asarray(inputs["w3"], np.float32),
        np.asarray(inputs["b3"], np.float32),
        np.asarray(inputs["w4"], np.float32),
        np.asarray(inputs["b4"], np.float32),
        np.asarray(inputs["w6"], np.float32),
        np.asarray(inputs["b6"], np.float32),
    )
    lhsT = _bf16(lhsT)
    nc = _build_module(reps=reps)

    in_maps = [
        {
            "x": np.ascontiguousarray(
                x[:, :, _B_PER_CORE * i : _B_PER_CORE * (i + 1), :]
            ),
            "lhsT": lhsT,
            "biasv": biasv,
        }
        for i in range(_N_CORES)
    ]
    res = run_bass_kernel_spmd(
        nc, in_maps, core_ids=list(range(_N_CORES)), trace=trace
    )
    out = np.empty((128, 16, _HO, _WO), np.float32)
    for i in range(_N_CORES):
        yi = np.asarray(res.results[i]["y"])  # [16o, 252h, 16img, 256w] bf16
        out[16 * i : 16 * (i + 1)] = yi.transpose(2, 0, 1, 3)[:, :, :, :_WO]
    return out, res


def kernel(**inputs):
    out, _ = _run(inputs, trace=False)
    return out
